# revision 19
# baseline (speedup 1.0000x reference)
"""Trainium2 Bass kernel for 2-hop MixHop GCN (nn_Mixhop).

Strategy (8 NeuronCores, node sharding):
  h = x @ W1 (+b1);  GCN norm folded into row scales:
      g = dinv * h;  y[d] = dinv[d] * sum_{e: src->d} g[src]
  Per hop: AllGather fp16 g-table across cores, per-edge dma_gather of
  source rows (table split in two 32K-row halves for int16 indices),
  segment-sum via PE matmuls with host-built one-hot fp8 "S" matrices
  (PSUM accumulation per 128-dst window).  Self-loop edges are excluded
  from the gather stream: each window's self contribution is one
  sequential 32KB DMA from the local gin table + an identity matmul.
  Gather calls are per (window, half) with per-core runtime counts
  (num_idxs_reg) so cross-core padding slots cost no DMA packets.
  relu'd mats are PE-transposed into matsT for the final lin2 (@W2)
  + log_softmax.
"""

import os
import sys

sys.path.insert(0, "/opt/trn_rl_repo")

import numpy as np

import concourse.bacc as bacc
import concourse.bass as bass
import concourse.mybir as mybir
import concourse.tile as tile
from concourse.bass_utils import run_bass_kernel_spmd

F32 = mybir.dt.float32
F16 = mybir.dt.float16
FP8 = mybir.dt.float8e4
I16 = mybir.dt.int16
U32 = mybir.dt.uint32
NP_FP8 = mybir.dt.np(FP8)
NP_F16 = np.float16

N_CORES = 8
WIN = 128          # dst nodes per PSUM window
CHUNK = 128        # edges per matmul chunk
NGBUF = 12         # G-tile ring depth (windows in flight)

LAST_EXEC_NS = None
LAST_RESULTS = None


def _preprocess(x, edge_index, W1, b1, W2, b2):
    """Build the chunk plan (program-level constants, max over cores) and
    per-core input arrays."""
    n_nodes, d_in = x.shape
    hid = W1.shape[1]
    ncls = W2.shape[1]
    nmat = W2.shape[0] // hid
    assert n_nodes % (N_CORES * WIN) == 0
    NLOC = n_nodes // N_CORES
    HALF = n_nodes // 2
    NW = NLOC // WIN
    KIN = d_in // 128
    assert d_in % 128 == 0 and hid == 128

    src = np.asarray(edge_index[0], dtype=np.int64)
    dst = np.asarray(edge_index[1], dtype=np.int64)

    # degree includes the self loop (reference appends one per node)
    deg = (np.bincount(dst, minlength=n_nodes) + 1).astype(np.float32)
    dinv = (1.0 / np.sqrt(deg)).astype(np.float32)

    core = dst // NLOC
    w_of = (dst % NLOC) // WIN
    # table halves hold local-row ranges [0,NLOC/2) / [NLOC/2,NLOC) of every
    # core (peer-major within a half), so each half AllGather output is one
    # contiguous Shared tensor with a single writer
    HROWS = NLOC // 2
    src_p = src // NLOC
    src_r = src % NLOC
    half_of = (src_r >= HROWS).astype(np.int64)
    srcrow = (src_p * HROWS + src_r % HROWS).astype(np.int64)
    dloc = (dst % WIN).astype(np.int64)

    # counts per (core, window, half) -> program chunk counts = max over cores
    key = (core * NW + w_of) * 2 + half_of
    cnt = np.bincount(key, minlength=N_CORES * NW * 2).reshape(N_CORES, NW, 2)
    chunks_pc = -(-cnt // CHUNK)  # ceil-div per core
    C = chunks_pc.max(axis=0)     # [NW, 2] max over cores
    CL, CH = C[:, 0].copy(), C[:, 1].copy()
    CW = CL + CH

    # per-window slot layout: [L chunks | H chunks]
    wbase = np.concatenate([[0], np.cumsum(CW)[:-1]])  # col base per window
    TOTC = int(CW.sum())
    slotbase = np.zeros((NW, 2), np.int64)
    for w in range(NW):
        slotbase[w, 0] = wbase[w] * CHUNK
        slotbase[w, 1] = (wbase[w] + CL[w]) * CHUNK
    TOTSLOTS = TOTC * CHUNK

    # S data col base per window (same layout as slots)
    soff = wbase * CHUNK
    CMAXW = int(CW.max())

    one_fp8 = np.float32(1.0).astype(NP_FP8).view(np.uint8)

    plan = dict(
        n_nodes=n_nodes, NLOC=NLOC, HALF=HALF, NW=NW, KIN=KIN,
        hid=hid, ncls=ncls, nmat=nmat,
        CL=CL, CH=CH, CW=CW, wbase=wbase, soff=soff,
        TOTC=TOTC, TOTSLOTS=TOTSLOTS, CMAXW=CMAXW,
        has_b1=bool(np.any(b1 != 0)), has_b2=bool(np.any(b2 != 0)),
    )

    in_maps = []
    for p in range(N_CORES):
        sel = core == p
        s_p, w_p, h_p, dl_p = srcrow[sel], w_of[sel], half_of[sel], dloc[sel]
        k = w_p * 2 + h_p
        order = np.argsort(k, kind="stable")
        ks = k[order]
        gcnt = np.bincount(ks, minlength=NW * 2)
        run_start = np.cumsum(gcnt) - gcnt
        run_pos = np.arange(len(ks)) - np.repeat(run_start, gcnt)
        slots = slotbase.reshape(-1)[ks] + run_pos

        idx_flat = np.zeros(TOTSLOTS, np.int16)
        idx_flat[slots] = s_p[order].astype(np.int16)
        idx16 = idx_flat.reshape(TOTSLOTS // 16, 16).T  # [16, S/16]
        idx_arr = np.tile(idx16, (8, 1)).copy()         # [128, S/16]

        # S one-hot: row = pos-in-chunk, col = window-S-col
        su8 = np.zeros((CHUNK, TOTC * CHUNK), np.uint8)
        c_in_list = run_pos // CHUNK
        pos = run_pos % CHUNK
        w_o = w_p[order]
        scol = (soff[w_o] + (c_in_list + np.where(h_p[order] == 1, CL[w_o], 0))
                * CHUNK + dl_p[order])
        su8[pos, scol] = one_fp8
        s_arr = su8.view(NP_FP8)

        x_p = np.asarray(x[p * NLOC:(p + 1) * NLOC], dtype=np.float32)
        xt = np.ascontiguousarray(
            x_p.reshape(NW, 128, KIN, 128).transpose(0, 3, 2, 1)
            .reshape(NW, 128, KIN * 128))
        dinv_p = np.ascontiguousarray(
            dinv[p * NLOC:(p + 1) * NLOC].reshape(NW, 128).T)

        m = {
            "xt": xt.astype(NP_F16),
            "w1": np.ascontiguousarray(
                np.asarray(W1, np.float32).reshape(KIN, 128, hid)
                .transpose(1, 0, 2).reshape(128, KIN * hid)).astype(NP_F16),
            "w2": np.ascontiguousarray(
                np.asarray(W2, np.float32).reshape(nmat, hid, ncls)
                .astype(NP_F16).transpose(1, 0, 2).reshape(hid, nmat * ncls)),
            "dinv": dinv_p,
            "dinv2": (dinv_p * dinv_p),
            "idx": idx_arr,
            "sdat": s_arr,
            "ident": np.eye(128, dtype=NP_F16),
        }
        if plan["has_b1"]:
            m["b1bc"] = np.tile(np.asarray(b1, np.float32)[None, :], (128, 1))
        if plan["has_b2"]:
            m["b2bc"] = np.tile(np.asarray(b2, np.float32)[None, :], (128, 1))
        in_maps.append(m)
    return plan, in_maps


def _build(plan):
    P = plan
    NLOC, NW, KIN = P["NLOC"], P["NW"], P["KIN"]
    HID, NCLS, NMAT = P["hid"], P["ncls"], P["nmat"]
    HALF, NN = P["HALF"], P["n_nodes"]
    CL, CH, CW = P["CL"], P["CH"], P["CW"]
    wbase, soff = P["wbase"], P["soff"]
    CMAXW, TOTC, TOTSLOTS = P["CMAXW"], P["TOTC"], P["TOTSLOTS"]

    nc = bacc.Bacc("TRN2", target_bir_lowering=False, debug=False,
                   num_devices=N_CORES, num_swdge_queues=4)
    xt_d = nc.dram_tensor("xt", [NW, 128, KIN * 128], F16,
                          kind="ExternalInput")
    w1_d = nc.dram_tensor("w1", [128, KIN * HID], F16, kind="ExternalInput")
    w2_d = nc.dram_tensor("w2", [128, NMAT * NCLS], F16, kind="ExternalInput")
    dinv_d = nc.dram_tensor("dinv", [128, NW], F32, kind="ExternalInput")
    dinv2_d = nc.dram_tensor("dinv2", [128, NW], F32, kind="ExternalInput")
    idx_d = nc.dram_tensor("idx", [128, TOTSLOTS // 16], I16,
                           kind="ExternalInput")
    sdat_d = nc.dram_tensor("sdat", [128, TOTC * CHUNK], FP8,
                            kind="ExternalInput")
    id_d = nc.dram_tensor("ident", [128, 128], F16, kind="ExternalInput")
    b1_d = (nc.dram_tensor("b1bc", [128, HID], F32, kind="ExternalInput")
            if P["has_b1"] else None)
    b2_d = (nc.dram_tensor("b2bc", [128, NCLS], F32, kind="ExternalInput")
            if P["has_b2"] else None)
    y_d = nc.dram_tensor("y", [NLOC, NCLS], F32, kind="ExternalOutput")

    rg = [list(range(N_CORES))]

    with tile.TileContext(nc) as tc:
        # ---- persistent tiles ----
        perm = tc.alloc_tile_pool(name="perm", bufs=1)
        dramp = tc.alloc_tile_pool(name="dramp", bufs=1, space="DRAM")
        w1_sb = perm.tile([128, KIN * HID], F16, name="w1sb")
        w2_sb = perm.tile([128, NMAT * NCLS], F16, name="w2sb")
        dinv_sb = perm.tile([128, NW], F32, name="dinvsb")
        dinv2_sb = perm.tile([128, NW], F32, name="dinv2sb")
        idx_sb = perm.tile([128, TOTSLOTS // 16], I16, name="idxsb")
        id_sb = perm.tile([128, 128], F16, name="idsb")
        matsT = [perm.tile([128, NLOC], F16, name=f"matsT{i}")
                 for i in range(NMAT)]
        logits = perm.tile([128, NW * NCLS], F32, name="logits")
        epack = perm.tile([128, NW * NCLS], F32, name="epack")
        ssum = perm.tile([128, NW], F32, name="ssum")
        lsum = perm.tile([128, NW], F32, name="lsum")
        final = perm.tile([128, NW * NCLS], F32, name="final")
        b1_sb = perm.tile([128, HID], F32, name="b1sb") if b1_d else None
        b2_sb = perm.tile([128, NCLS], F32, name="b2sb") if b2_d else None
        # persistent G ring: gather targets, memset once (skipped pad slots
        # must hold finite values: 0 * NaN would poison the PSUM sum)
        G_ring = [perm.tile([128, CMAXW * 128], F16, name=f"Gr{i}")
                  for i in range(NGBUF)]

        gin = [dramp.tile([NLOC, HID], F16, name=f"gin{h}")
               for h in range(2)]
        gout = [[dramp.tile([NN // 2, HID], F16, addr_space="Shared",
                            name=f"gout{h}_{hf}") for hf in range(2)]
                for h in range(2)]

        nc.sync.dma_start(out=w1_sb[:], in_=w1_d[:])
        nc.sync.dma_start(out=w2_sb[:], in_=w2_d[:])
        nc.sync.dma_start(out=dinv_sb[:], in_=dinv_d[:])
        nc.sync.dma_start(out=dinv2_sb[:], in_=dinv2_d[:])
        nc.scalar.dma_start(out=idx_sb[:], in_=idx_d[:])
        nc.sync.dma_start(out=id_sb[:], in_=id_d[:])
        if b1_d is not None:
            nc.sync.dma_start(out=b1_sb[:], in_=b1_d[:])
        if b2_d is not None:
            nc.sync.dma_start(out=b2_sb[:], in_=b2_d[:])
        for t in G_ring:
            nc.vector.memset(t[:], 0.0)

        with (
            tc.tile_pool(name="xp", bufs=6) as xp,
            tc.tile_pool(name="gsp", bufs=6) as gsp,
            tc.tile_pool(name="sp", bufs=8) as sp,
            tc.tile_pool(name="dp", bufs=6) as dp,
            tc.tile_pool(name="pp", bufs=1, space="PSUM") as pp,
        ):
            ACT = mybir.ActivationFunctionType

            def drain_window(acc, w, hop):
                """acc: PSUM [128, HID] f32 for window w; hop 0/1/-1 (lin1).

                lin1: h = acc.  hops: h = dinv * acc (the segment sum still
                needs the dst-side dinv).  g-table for next hop = dinv * h.
                mats = relu(h)."""
                hscale = dinv_sb[:, w:w + 1] if hop >= 0 else 1.0
                gscale = (dinv2_sb[:, w:w + 1] if hop >= 0
                          else dinv_sb[:, w:w + 1])
                if hop < 1:  # produce g for the next AllGather
                    gt = dp.tile([128, HID], F16, tag="gt")
                    nc.scalar.activation(gt[:], acc[:], ACT.Copy, scale=gscale)
                    nc.sync.dma_start(
                        out=gin[hop + 1][w * 128:(w + 1) * 128, :], in_=gt[:])
                m = dp.tile([128, HID], F16, tag="m")
                nc.scalar.activation(m[:], acc[:], ACT.Relu, scale=hscale)
                tp = pp.tile([128, 128], F16, tag="tp", bufs=2)
                nc.tensor.transpose(tp[:], m[:], id_sb[:])
                nc.scalar.activation(matsT[hop + 1][:, w * 128:(w + 1) * 128],
                                     tp[:], ACT.Copy)

            SLICE = NLOC // 2

            def ag_slice(hop, sl):
                nc.gpsimd.collective_compute(
                    "AllGather", mybir.AluOpType.bypass, replica_groups=rg,
                    ins=[gin[hop][sl * SLICE:(sl + 1) * SLICE, :]],
                    outs=[gout[hop][sl][:]])

            WPS = NW // 2  # windows per table slice

            # ---- lin1 ----
            for t in range(NW):
                acc = pp.tile([128, HID], F32, tag="acc", bufs=4)
                xtile = xp.tile([128, KIN * 128], F16, tag="xt")
                nc.sync.dma_start(out=xtile[:], in_=xt_d[t])
                for k in range(KIN):
                    nc.tensor.matmul(acc[:], xtile[:, k * 128:(k + 1) * 128],
                                     w1_sb[:, k * HID:(k + 1) * HID],
                                     start=(k == 0), stop=(k == KIN - 1))
                if b1_sb is not None:
                    hb = dp.tile([128, HID], F32, tag="hb")
                    nc.vector.tensor_tensor(hb[:], acc[:], b1_sb[:],
                                            op=mybir.AluOpType.add)
                    drain_window(hb, t, -1)
                else:
                    drain_window(acc, t, -1)
                if not os.environ.get("MIXHOP_NO_CC", "0") == "1" \
                        and t % WPS == WPS - 1:
                    ag_slice(0, t // WPS)

            def lin2_tile(t):
                lg = pp.tile([128, NCLS], F32, tag="lg", bufs=2)
                for mi in range(NMAT):
                    nc.tensor.matmul(lg[:], matsT[mi][:, t * 128:(t + 1) * 128],
                                     w2_sb[:, mi * NCLS:(mi + 1) * NCLS],
                                     start=(mi == 0), stop=(mi == NMAT - 1))
                dst = logits[:, t * NCLS:(t + 1) * NCLS]
                if b2_sb is not None:
                    nc.vector.tensor_tensor(dst, lg[:], b2_sb[:],
                                            op=mybir.AluOpType.add)
                else:
                    nc.vector.tensor_copy(dst, lg[:])

            # ---- hops ----
            no_cc = os.environ.get("MIXHOP_NO_CC", "0") == "1"
            no_gather = os.environ.get("MIXHOP_NO_GATHER", "0") == "1"
            qc = 0
            for hop in range(2):
                if no_cc:
                    for hf in range(2):
                        nc.sync.dma_start(
                            out=gout[hop][hf][0:SLICE, :],
                            in_=gin[hop][hf * SLICE:(hf + 1) * SLICE, :])
                table = gout[hop]
                for w in range(NW):
                    cl, ch, cw = int(CL[w]), int(CH[w]), int(CW[w])
                    G = G_ring[w % NGBUF]
                    G3 = G[:].rearrange("p (c e) -> p c e", e=128)
                    base = int(wbase[w]) * CHUNK
                    if no_gather:
                        pass
                    else:
                        # one gather call per (window, half)
                        for hi, (c0, ncols) in enumerate(((0, cl), (cl, ch))):
                            if not ncols:
                                continue
                            tab_ap = table[hi][:]
                            s0 = base + c0 * CHUNK
                            nc.gpsimd.dma_gather(
                                G3[:, c0:c0 + ncols, :], tab_ap,
                                idx_sb[:, s0 // 16:(s0 + ncols * CHUNK) // 16],
                                ncols * CHUNK, ncols * CHUNK, HID,
                                single_packet=False,
                                queue_num=qc % 4)
                            qc += 1
                    # self-loop contribution: sequential 32KB read of the
                    # window's own g rows from the local gin table, summed
                    # in via an identity matmul (fp16 I is exact)
                    gs = gsp.tile([128, HID], F16, tag="gs")
                    nc.sync.dma_start(
                        out=gs[:],
                        in_=gin[hop][w * 128:(w + 1) * 128, :])
                    S = sp.tile([128, CMAXW * 128], FP8, tag="S")
                    if cw:
                        nc.scalar.dma_start(
                            out=S[:, :cw * 128],
                            in_=sdat_d[:, int(soff[w]):int(soff[w])
                                       + cw * 128])
                    acc = pp.tile([128, HID], F32, tag="acc", bufs=4)
                    nc.tensor.matmul(acc[:], id_sb[:], gs[:],
                                     start=True, stop=(cw == 0))
                    for ci in range(cw):
                        nc.tensor.matmul(
                            acc[:], S[:, ci * 128:(ci + 1) * 128],
                            G3[:, ci, :],
                            start=False, stop=(ci == cw - 1))
                    drain_window(acc, w, hop)
                    if hop == 1:
                        lin2_tile(w)
                    elif not no_cc:
                        # fire next hop's table slices as soon as their
                        # producing windows have drained (margin NGBUF+1
                        # keeps the collective trigger from stalling the
                        # gather stream at the Pool engine queue head)
                        for sl in range(2):
                            due = min(sl * WPS + WPS - 1 + NGBUF + 1, NW - 1)
                            if w == due:
                                ag_slice(1, sl)

            # ---- log_softmax (lin2 already streamed into hop-1 drains) ----
            nc.scalar.activation(epack[:], logits[:],
                                 mybir.ActivationFunctionType.Exp)
            nc.vector.reduce_sum(
                ssum[:], epack[:].rearrange("p (t c) -> p t c", c=NCLS),
                axis=mybir.AxisListType.X)
            nc.scalar.activation(lsum[:], ssum[:],
                                 mybir.ActivationFunctionType.Ln)
            for t in range(NW):
                nc.vector.tensor_scalar_sub(
                    final[:, t * NCLS:(t + 1) * NCLS],
                    logits[:, t * NCLS:(t + 1) * NCLS], lsum[:, t:t + 1])
            nc.sync.dma_start(
                out=y_d[:].rearrange("(t q) c -> q t c", q=128),
                in_=final[:].rearrange("p (t c) -> p t c", c=NCLS))
        perm.release()
        dramp.release()
    nc.compile()
    return nc


def _ensure_ntff_hook():
    """The agent image's antenv lacks axon_hooks; synthesize it so
    run_bass_kernel_spmd(trace=True) can NTFF-profile via the axon .so."""
    import types

    if "antenv.axon_hooks" in sys.modules:
        return
    try:
        from trn_agent_boot.trn_boot import _ntff_profile_via_ctypes
        hook = _ntff_profile_via_ctypes("/opt/axon/libaxon_pjrt.so")
    except Exception:
        hook = None
    mod = types.ModuleType("antenv.axon_hooks")
    mod.get_axon_ntff_profile_hook = lambda: hook
    mod.set_axon_ntff_profile_hook = lambda h: None
    sys.modules["antenv.axon_hooks"] = mod


def kernel(x, edge_index, W1, b1, W2, b2):
    global LAST_EXEC_NS, LAST_RESULTS
    plan, in_maps = _preprocess(x, edge_index, W1, b1, W2, b2)
    nc = _build(plan)
    trace = os.environ.get("MIXHOP_TRACE", "0") == "1"
    if trace:
        _ensure_ntff_hook()
    res = run_bass_kernel_spmd(nc, in_maps, list(range(N_CORES)), trace=trace)
    LAST_EXEC_NS = res.exec_time_ns
    LAST_RESULTS = res
    out = np.concatenate([res.results[p]["y"] for p in range(N_CORES)], axis=0)
    return out.astype(np.float32)


# revision 26
# speedup vs baseline: 1.6807x; 1.6807x over previous
"""Trainium2 Bass kernel for 2-hop MixHop GCN (nn_Mixhop).

Strategy (8 NeuronCores, node sharding):
  h = x @ W1 (+b1);  GCN norm folded into row scales:
      g = dinv * h;  y[d] = dinv[d] * sum_{e: src->d} g[src]
  Per hop: AllGather fp16 g-table across cores, per-edge dma_gather of
  source rows (table split in two 32K-row halves for int16 indices),
  segment-sum via PE matmuls with host-built one-hot fp8 "S" matrices
  (PSUM accumulation per 128-dst window).  Self-loop edges are excluded
  from the gather stream: each window's self contribution is one
  sequential 32KB DMA from the local gin table + an identity matmul.
  Gather calls are per (window, half) with per-core runtime counts
  (num_idxs_reg) so cross-core padding slots cost no DMA packets.
  relu'd mats are PE-transposed into matsT for the final lin2 (@W2)
  + log_softmax.
"""

import os
import sys

sys.path.insert(0, "/opt/trn_rl_repo")

import numpy as np

import concourse.bacc as bacc
import concourse.bass as bass
import concourse.mybir as mybir
import concourse.tile as tile
from concourse.bass_utils import run_bass_kernel_spmd

F32 = mybir.dt.float32
F16 = mybir.dt.float16
FP8 = mybir.dt.float8e4
I16 = mybir.dt.int16
U32 = mybir.dt.uint32
NP_FP8 = mybir.dt.np(FP8)
NP_F16 = np.float16

N_CORES = 8
WIN = 128          # dst nodes per PSUM window
CHUNK = 128        # edges per matmul chunk
NGBUF = 12         # G-tile ring depth (windows in flight)

LAST_EXEC_NS = None
LAST_RESULTS = None


def _preprocess(x, edge_index, W1, b1, W2, b2):
    """Build the chunk plan (program-level constants, max over cores) and
    per-core input arrays."""
    n_nodes, d_in = x.shape
    hid = W1.shape[1]
    ncls = W2.shape[1]
    nmat = W2.shape[0] // hid
    assert n_nodes % (N_CORES * WIN) == 0
    NLOC = n_nodes // N_CORES
    HALF = n_nodes // 2
    NW = NLOC // WIN
    KIN = d_in // 128
    assert d_in % 128 == 0 and hid == 128

    src = np.asarray(edge_index[0], dtype=np.int64)
    dst = np.asarray(edge_index[1], dtype=np.int64)

    # degree includes the self loop (reference appends one per node)
    deg = (np.bincount(dst, minlength=n_nodes) + 1).astype(np.float32)
    dinv = (1.0 / np.sqrt(deg)).astype(np.float32)

    core = dst // NLOC
    w_of = (dst % NLOC) // WIN
    # table halves hold local-row ranges [0,NLOC/2) / [NLOC/2,NLOC) of every
    # core (peer-major within a half), so each half AllGather output is one
    # contiguous Shared tensor with a single writer
    HROWS = NLOC // 2
    src_p = src // NLOC
    src_r = src % NLOC
    half_of = (src_r >= HROWS).astype(np.int64)
    srcrow = (src_p * HROWS + src_r % HROWS).astype(np.int64)
    dloc = (dst % WIN).astype(np.int64)

    # counts per (core, window, half) -> program chunk counts = max over cores
    key = (core * NW + w_of) * 2 + half_of
    cnt = np.bincount(key, minlength=N_CORES * NW * 2).reshape(N_CORES, NW, 2)
    chunks_pc = -(-cnt // CHUNK)  # ceil-div per core
    C = chunks_pc.max(axis=0)     # [NW, 2] max over cores
    CL, CH = C[:, 0].copy(), C[:, 1].copy()
    CW = CL + CH

    # per-window slot layout: [L chunks | H chunks]
    wbase = np.concatenate([[0], np.cumsum(CW)[:-1]])  # col base per window
    TOTC = int(CW.sum())
    slotbase = np.zeros((NW, 2), np.int64)
    for w in range(NW):
        slotbase[w, 0] = wbase[w] * CHUNK
        slotbase[w, 1] = (wbase[w] + CL[w]) * CHUNK
    TOTSLOTS = TOTC * CHUNK

    # S data col base per window (same layout as slots)
    soff = wbase * CHUNK
    CMAXW = int(CW.max())

    one_fp8 = np.float32(1.0).astype(NP_FP8).view(np.uint8)

    plan = dict(
        n_nodes=n_nodes, NLOC=NLOC, HALF=HALF, NW=NW, KIN=KIN,
        hid=hid, ncls=ncls, nmat=nmat,
        CL=CL, CH=CH, CW=CW, wbase=wbase, soff=soff,
        TOTC=TOTC, TOTSLOTS=TOTSLOTS, CMAXW=CMAXW,
        has_b1=bool(np.any(b1 != 0)), has_b2=bool(np.any(b2 != 0)),
    )

    in_maps = []
    for p in range(N_CORES):
        sel = core == p
        s_p, w_p, h_p, dl_p = srcrow[sel], w_of[sel], half_of[sel], dloc[sel]
        k = w_p * 2 + h_p
        order = np.argsort(k, kind="stable")
        ks = k[order]
        gcnt = np.bincount(ks, minlength=NW * 2)
        run_start = np.cumsum(gcnt) - gcnt
        run_pos = np.arange(len(ks)) - np.repeat(run_start, gcnt)
        slots = slotbase.reshape(-1)[ks] + run_pos

        idx_flat = np.zeros(TOTSLOTS, np.int16)
        idx_flat[slots] = s_p[order].astype(np.int16)
        idx16 = idx_flat.reshape(TOTSLOTS // 16, 16).T  # [16, S/16]
        idx_arr = np.tile(idx16, (8, 1)).copy()         # [128, S/16]

        # S one-hot: row = pos-in-chunk, col = window-S-col
        su8 = np.zeros((CHUNK, TOTC * CHUNK), np.uint8)
        c_in_list = run_pos // CHUNK
        pos = run_pos % CHUNK
        w_o = w_p[order]
        scol = (soff[w_o] + (c_in_list + np.where(h_p[order] == 1, CL[w_o], 0))
                * CHUNK + dl_p[order])
        su8[pos, scol] = one_fp8
        s_arr = su8.view(NP_FP8)

        x_p = np.asarray(x[p * NLOC:(p + 1) * NLOC], dtype=np.float32)
        xt = np.ascontiguousarray(
            x_p.reshape(NW, 128, KIN, 128).transpose(0, 3, 2, 1)
            .reshape(NW, 128, KIN * 128))
        dinv_p = np.ascontiguousarray(
            dinv[p * NLOC:(p + 1) * NLOC].reshape(NW, 128).T)

        m = {
            "xt": xt.astype(mybir.dt.np(FP8)),
            "w1": np.ascontiguousarray(
                np.asarray(W1, np.float32).reshape(KIN, 128, hid)
                .transpose(1, 0, 2).reshape(128, KIN * hid)).astype(NP_F16),
            "w2": np.ascontiguousarray(
                np.asarray(W2, np.float32).reshape(nmat, hid, ncls)
                .astype(NP_F16).transpose(1, 0, 2).reshape(hid, nmat * ncls)),
            "dinv": dinv_p,
            "dinv2": (dinv_p * dinv_p),
            "idx": idx_arr,
            "sdat": s_arr,
            "ident": np.eye(128, dtype=NP_F16),
        }
        if plan["has_b1"]:
            m["b1bc"] = np.tile(np.asarray(b1, np.float32)[None, :], (128, 1))
        if plan["has_b2"]:
            m["b2bc"] = np.tile(np.asarray(b2, np.float32)[None, :], (128, 1))
        in_maps.append(m)
    return plan, in_maps


def _build(plan):
    P = plan
    NLOC, NW, KIN = P["NLOC"], P["NW"], P["KIN"]
    HID, NCLS, NMAT = P["hid"], P["ncls"], P["nmat"]
    HALF, NN = P["HALF"], P["n_nodes"]
    CL, CH, CW = P["CL"], P["CH"], P["CW"]
    wbase, soff = P["wbase"], P["soff"]
    CMAXW, TOTC, TOTSLOTS = P["CMAXW"], P["TOTC"], P["TOTSLOTS"]

    nc = bacc.Bacc("TRN2", target_bir_lowering=False, debug=False,
                   num_devices=N_CORES, num_swdge_queues=4)
    xt_d = nc.dram_tensor("xt", [NW, 128, KIN * 128], FP8,
                          kind="ExternalInput")
    w1_d = nc.dram_tensor("w1", [128, KIN * HID], F16, kind="ExternalInput")
    w2_d = nc.dram_tensor("w2", [128, NMAT * NCLS], F16, kind="ExternalInput")
    dinv_d = nc.dram_tensor("dinv", [128, NW], F32, kind="ExternalInput")
    dinv2_d = nc.dram_tensor("dinv2", [128, NW], F32, kind="ExternalInput")
    idx_d = nc.dram_tensor("idx", [128, TOTSLOTS // 16], I16,
                           kind="ExternalInput")
    sdat_d = nc.dram_tensor("sdat", [128, TOTC * CHUNK], FP8,
                            kind="ExternalInput")
    id_d = nc.dram_tensor("ident", [128, 128], F16, kind="ExternalInput")
    b1_d = (nc.dram_tensor("b1bc", [128, HID], F32, kind="ExternalInput")
            if P["has_b1"] else None)
    b2_d = (nc.dram_tensor("b2bc", [128, NCLS], F32, kind="ExternalInput")
            if P["has_b2"] else None)
    y_d = nc.dram_tensor("y", [NLOC, NCLS], F32, kind="ExternalOutput")

    rg = [list(range(N_CORES))]

    with tile.TileContext(nc) as tc:
        # ---- persistent tiles ----
        perm = tc.alloc_tile_pool(name="perm", bufs=1)
        dramp = tc.alloc_tile_pool(name="dramp", bufs=1, space="DRAM")
        w1_sb = perm.tile([128, KIN * HID], F16, name="w1sb")
        w2_sb = perm.tile([128, NMAT * NCLS], F16, name="w2sb")
        dinv_sb = perm.tile([128, NW], F32, name="dinvsb")
        dinv2_sb = perm.tile([128, NW], F32, name="dinv2sb")
        idx_sb = perm.tile([128, TOTSLOTS // 16], I16, name="idxsb")
        id_sb = perm.tile([128, 128], F16, name="idsb")
        matsT = [perm.tile([128, NLOC], F16, name=f"matsT{i}")
                 for i in range(NMAT)]
        logits = perm.tile([128, NW * NCLS], F32, name="logits")
        epack = perm.tile([128, NW * NCLS], F32, name="epack")
        ssum = perm.tile([128, NW], F32, name="ssum")
        lsum = perm.tile([128, NW], F32, name="lsum")
        final = perm.tile([128, NW * NCLS], F32, name="final")
        b1_sb = perm.tile([128, HID], F32, name="b1sb") if b1_d else None
        b2_sb = perm.tile([128, NCLS], F32, name="b2sb") if b2_d else None
        # persistent G ring: gather targets, memset once (skipped pad slots
        # must hold finite values: 0 * NaN would poison the PSUM sum)
        G_ring = [perm.tile([128, CMAXW * 128], F16, name=f"Gr{i}")
                  for i in range(NGBUF)]

        gin = [dramp.tile([NLOC, HID], F16, name=f"gin{h}")
               for h in range(2)]
        gout = [[dramp.tile([NN // 2, HID], F16, addr_space="Shared",
                            name=f"gout{h}_{hf}") for hf in range(2)]
                for h in range(2)]

        nc.sync.dma_start(out=w1_sb[:], in_=w1_d[:])
        nc.sync.dma_start(out=w2_sb[:], in_=w2_d[:])
        nc.sync.dma_start(out=dinv_sb[:], in_=dinv_d[:])
        nc.sync.dma_start(out=dinv2_sb[:], in_=dinv2_d[:])
        nc.sync.dma_start(out=idx_sb[:], in_=idx_d[:])
        nc.sync.dma_start(out=id_sb[:], in_=id_d[:])
        if b1_d is not None:
            nc.sync.dma_start(out=b1_sb[:], in_=b1_d[:])
        if b2_d is not None:
            nc.sync.dma_start(out=b2_sb[:], in_=b2_d[:])
        for t in G_ring:
            nc.vector.memset(t[:], 0.0)

        with (
            tc.tile_pool(name="xp", bufs=6) as xp,
            tc.tile_pool(name="gsp", bufs=6) as gsp,
            tc.tile_pool(name="sp", bufs=8) as sp,
            tc.tile_pool(name="dp", bufs=6) as dp,
            tc.tile_pool(name="pp", bufs=1, space="PSUM") as pp,
        ):
            ACT = mybir.ActivationFunctionType

            def drain_window(acc, w, hop):
                """acc: PSUM [128, HID] f32 for window w; hop 0/1/-1 (lin1).

                lin1: h = acc.  hops: h = dinv * acc (the segment sum still
                needs the dst-side dinv).  g-table for next hop = dinv * h.
                mats = relu(h)."""
                hscale = dinv_sb[:, w:w + 1] if hop >= 0 else 1.0
                gscale = (dinv2_sb[:, w:w + 1] if hop >= 0
                          else dinv_sb[:, w:w + 1])
                if hop < 1:  # produce g for the next AllGather
                    gt = dp.tile([128, HID], F16, tag="gt")
                    nc.scalar.activation(gt[:], acc[:], ACT.Copy, scale=gscale)
                    nc.sync.dma_start(
                        out=gin[hop + 1][w * 128:(w + 1) * 128, :], in_=gt[:])
                m = dp.tile([128, HID], F16, tag="m")
                nc.scalar.activation(m[:], acc[:], ACT.Relu, scale=hscale)
                tp = pp.tile([128, 128], F16, tag="tp", bufs=2)
                nc.tensor.transpose(tp[:], m[:], id_sb[:])
                nc.scalar.activation(matsT[hop + 1][:, w * 128:(w + 1) * 128],
                                     tp[:], ACT.Copy)

            SLICE = NLOC // 2

            def ag_slice(hop, sl):
                nc.gpsimd.collective_compute(
                    "AllGather", mybir.AluOpType.bypass, replica_groups=rg,
                    ins=[gin[hop][sl * SLICE:(sl + 1) * SLICE, :]],
                    outs=[gout[hop][sl][:]])

            WPS = NW // 2  # windows per table slice

            # ---- lin1 ----
            for t in range(NW):
                acc = pp.tile([128, HID], F32, tag="acc", bufs=4)
                xtile = xp.tile([128, KIN * 128], FP8, tag="xt")
                nc.sync.dma_start(out=xtile[:], in_=xt_d[t])
                for k in range(KIN):
                    nc.tensor.matmul(acc[:], xtile[:, k * 128:(k + 1) * 128],
                                     w1_sb[:, k * HID:(k + 1) * HID],
                                     start=(k == 0), stop=(k == KIN - 1))
                if b1_sb is not None:
                    hb = dp.tile([128, HID], F32, tag="hb")
                    nc.vector.tensor_tensor(hb[:], acc[:], b1_sb[:],
                                            op=mybir.AluOpType.add)
                    drain_window(hb, t, -1)
                else:
                    drain_window(acc, t, -1)
                if not os.environ.get("MIXHOP_NO_CC", "0") == "1" \
                        and t % WPS == WPS - 1:
                    ag_slice(0, t // WPS)

            def lin2_tile(t):
                lg = pp.tile([128, NCLS], F32, tag="lg", bufs=2)
                for mi in range(NMAT):
                    nc.tensor.matmul(lg[:], matsT[mi][:, t * 128:(t + 1) * 128],
                                     w2_sb[:, mi * NCLS:(mi + 1) * NCLS],
                                     start=(mi == 0), stop=(mi == NMAT - 1))
                dst = logits[:, t * NCLS:(t + 1) * NCLS]
                if b2_sb is not None:
                    nc.vector.tensor_tensor(dst, lg[:], b2_sb[:],
                                            op=mybir.AluOpType.add)
                else:
                    nc.vector.tensor_copy(dst, lg[:])

            # ---- hops ----
            no_cc = os.environ.get("MIXHOP_NO_CC", "0") == "1"
            no_gather = os.environ.get("MIXHOP_NO_GATHER", "0") == "1"
            qc = 0
            for hop in range(2):
                if no_cc:
                    for hf in range(2):
                        nc.sync.dma_start(
                            out=gout[hop][hf][0:SLICE, :],
                            in_=gin[hop][hf * SLICE:(hf + 1) * SLICE, :])
                table = gout[hop]
                for w in range(NW):
                    cl, ch, cw = int(CL[w]), int(CH[w]), int(CW[w])
                    G = G_ring[w % NGBUF]
                    G3 = G[:].rearrange("p (c e) -> p c e", e=128)
                    base = int(wbase[w]) * CHUNK
                    if no_gather:
                        pass
                    else:
                        # one gather call per (window, half)
                        for hi, (c0, ncols) in enumerate(((0, cl), (cl, ch))):
                            if not ncols:
                                continue
                            tab_ap = table[hi][:]
                            s0 = base + c0 * CHUNK
                            nc.gpsimd.dma_gather(
                                G3[:, c0:c0 + ncols, :], tab_ap,
                                idx_sb[:, s0 // 16:(s0 + ncols * CHUNK) // 16],
                                ncols * CHUNK, ncols * CHUNK, HID,
                                single_packet=True,
                                queue_num=qc % 4)
                            qc += 1
                    # self-loop contribution: sequential 32KB read of the
                    # window's own g rows from the local gin table, summed
                    # in via an identity matmul (fp16 I is exact)
                    gs = gsp.tile([128, HID], F16, tag="gs")
                    nc.sync.dma_start(
                        out=gs[:],
                        in_=gin[hop][w * 128:(w + 1) * 128, :])
                    S = sp.tile([128, CMAXW * 128], FP8, tag="S")
                    if cw:
                        nc.scalar.dma_start(
                            out=S[:, :cw * 128],
                            in_=sdat_d[:, int(soff[w]):int(soff[w])
                                       + cw * 128])
                    acc = pp.tile([128, HID], F32, tag="acc", bufs=4)
                    nc.tensor.matmul(acc[:], id_sb[:], gs[:],
                                     start=True, stop=(cw == 0))
                    for ci in range(cw):
                        nc.tensor.matmul(
                            acc[:], S[:, ci * 128:(ci + 1) * 128],
                            G3[:, ci, :],
                            start=False, stop=(ci == cw - 1))
                    drain_window(acc, w, hop)
                    if hop == 1:
                        lin2_tile(w)
                    elif not no_cc:
                        # fire next hop's table slices as soon as their
                        # producing windows have drained (margin NGBUF+1
                        # keeps the collective trigger from stalling the
                        # gather stream at the Pool engine queue head)
                        for sl in range(2):
                            due = min(sl * WPS + WPS - 1 + NGBUF + 1, NW - 1)
                            if w == due:
                                ag_slice(1, sl)

            # ---- log_softmax (lin2 already streamed into hop-1 drains) ----
            nc.scalar.activation(epack[:], logits[:],
                                 mybir.ActivationFunctionType.Exp)
            nc.vector.reduce_sum(
                ssum[:], epack[:].rearrange("p (t c) -> p t c", c=NCLS),
                axis=mybir.AxisListType.X)
            nc.scalar.activation(lsum[:], ssum[:],
                                 mybir.ActivationFunctionType.Ln)
            for t in range(NW):
                nc.vector.tensor_scalar_sub(
                    final[:, t * NCLS:(t + 1) * NCLS],
                    logits[:, t * NCLS:(t + 1) * NCLS], lsum[:, t:t + 1])
            nc.sync.dma_start(
                out=y_d[:].rearrange("(t q) c -> q t c", q=128),
                in_=final[:].rearrange("p (t c) -> p t c", c=NCLS))
        perm.release()
        dramp.release()
    nc.compile()
    return nc


def _ensure_ntff_hook():
    """The agent image's antenv lacks axon_hooks; synthesize it so
    run_bass_kernel_spmd(trace=True) can NTFF-profile via the axon .so."""
    import types

    if "antenv.axon_hooks" in sys.modules:
        return
    try:
        from trn_agent_boot.trn_boot import _ntff_profile_via_ctypes
        hook = _ntff_profile_via_ctypes("/opt/axon/libaxon_pjrt.so")
    except Exception:
        hook = None
    mod = types.ModuleType("antenv.axon_hooks")
    mod.get_axon_ntff_profile_hook = lambda: hook
    mod.set_axon_ntff_profile_hook = lambda h: None
    sys.modules["antenv.axon_hooks"] = mod


def kernel(x, edge_index, W1, b1, W2, b2):
    global LAST_EXEC_NS, LAST_RESULTS
    plan, in_maps = _preprocess(x, edge_index, W1, b1, W2, b2)
    nc = _build(plan)
    trace = os.environ.get("MIXHOP_TRACE", "0") == "1"
    if trace:
        _ensure_ntff_hook()
    res = run_bass_kernel_spmd(nc, in_maps, list(range(N_CORES)), trace=trace)
    LAST_EXEC_NS = res.exec_time_ns
    LAST_RESULTS = res
    out = np.concatenate([res.results[p]["y"] for p in range(N_CORES)], axis=0)
    return out.astype(np.float32)
